# revision 5
# baseline (speedup 1.0000x reference)
"""Trainium2 Bass kernel for EnhancedAttentionV2:
sliding-window (256) attention + residual + layernorm, B=1, S=4096, HS=1024,
H=16 heads, D=64. Sequence-parallel across 8 NeuronCores: each core computes
512 query rows; K/V for its 768-key band (128-halo each side) are recomputed
locally from a zero-padded transposed slice of hidden_states, so no
collectives are needed.

Per-core device algorithm (all matmuls in float32r):
  qT[dout, s]  = WqT-chunks.T @ hT        (dT-major, 64-row head slices)
  kT[dout, j]  = WkT-chunks.T @ hT_band
  v'[j, h*66+.]= hT-chunks.T @ WvT        (s-major, per-head 66-wide slot:
                                           64 v cols | kmask col | pad)
  per (head, key-chunk o):  scoresT[j, q] = kT-slice.T @ qT-slice  (one matmul,
        q spans the <=3 query blocks whose window covers chunk o)
  probsT = exp(0.125 * scoresT)           (no max-subtraction; scores ~ N(0,1))
  band masks: tri-mask multiply on the 2 edge slices of each chunk
  ctxT'[66, q] += v'-slot.T @ probsT      (row 64 = softmax denominators via
                                           the kmask column riding in lhsT)
  transpose ctxT' per query block (PE transpose), then
  res += ctx * recip(denom)               (fused un-normalize + residual)
  layernorm rows of res -> out
"""

import os
from contextlib import ExitStack

import numpy as np

import concourse.bass as bass
import concourse.mybir as mybir
import concourse.tile as tile
from concourse import bacc
from concourse.bass_utils import run_bass_kernel_spmd

P = 128
S, HS, H, D = 4096, 1024, 16, 64
N_CORES = 8
SL = S // N_CORES           # 512 local query rows
QB = SL // P                # 4 query blocks
SBAND = SL + 2 * P          # 768 band keys
OB = SBAND // P             # 6 key chunks
KC = HS // P                # 8 contraction chunks
VST = 66                    # per-head slot width in v' (64 v | kmask | pad)
EPS = 1e-12
F32 = mybir.dt.float32
F32R = mybir.dt.float32r
AF = mybir.ActivationFunctionType
ALU = mybir.AluOpType
AX = mybir.AxisListType


def _build(has_b: bool, has_ln: bool):
    nc = bacc.Bacc(None, target_bir_lowering=False, debug=False,
                   num_devices=N_CORES)
    hT_d = nc.dram_tensor("hT", [HS, SBAND], F32R, kind="ExternalInput").ap()
    wq_d = nc.dram_tensor("wqT", [HS, HS], F32R, kind="ExternalInput").ap()
    wk_d = nc.dram_tensor("wkT", [HS, HS], F32R, kind="ExternalInput").ap()
    wv_d = nc.dram_tensor("wvT", [HS, HS], F32R, kind="ExternalInput").ap()
    res_d = nc.dram_tensor("res", [SL, HS], F32, kind="ExternalInput").ap()
    msk_d = nc.dram_tensor("masks", [P, 2 * P], F32, kind="ExternalInput").ap()
    idn_d = nc.dram_tensor("iden", [P, P], F32, kind="ExternalInput").ap()
    km_d = nc.dram_tensor("kmask", [SBAND], F32, kind="ExternalInput").ap()
    if has_b:
        bq_d = nc.dram_tensor("bq", [HS], F32, kind="ExternalInput").ap()
        bk_d = nc.dram_tensor("bk", [HS], F32, kind="ExternalInput").ap()
        bv_d = nc.dram_tensor("bv", [1, HS], F32R, kind="ExternalInput").ap()
    if has_ln:
        gam_d = nc.dram_tensor("gam", [P, HS], F32, kind="ExternalInput").ap()
        bet_d = nc.dram_tensor("bet", [P, HS], F32, kind="ExternalInput").ap()
    out_d = nc.dram_tensor("out", [SL, HS], F32, kind="ExternalOutput").ap()

    hT_re = hT_d.rearrange("(c p) j -> c p j", p=P)
    wq_re = wq_d.rearrange("(c p) d -> c p d", p=P)
    wk_re = wk_d.rearrange("(c p) d -> c p d", p=P)
    wv_re = wv_d.rearrange("(c p) d -> c p d", p=P)

    with tile.TileContext(nc) as tc, ExitStack() as ctx:
        const = ctx.enter_context(tc.tile_pool(name="const", bufs=1))
        wpool = ctx.enter_context(tc.tile_pool(name="wpool", bufs=2))
        persist = ctx.enter_context(tc.tile_pool(name="persist", bufs=1))
        probs_p = ctx.enter_context(tc.tile_pool(name="probs", bufs=8))
        ctxsb_p = ctx.enter_context(tc.tile_pool(name="ctxsb", bufs=2))
        stats_p = ctx.enter_context(tc.tile_pool(name="stats", bufs=48))
        pp_proj = ctx.enter_context(tc.tile_pool(name="ppproj", bufs=2, space="PSUM"))
        pp_sc = ctx.enter_context(tc.tile_pool(name="ppsc", bufs=2, space="PSUM"))
        pp_ctx = ctx.enter_context(tc.tile_pool(name="ppctx", bufs=2, space="PSUM"))
        pp_tr = ctx.enter_context(tc.tile_pool(name="pptr", bufs=2, space="PSUM"))

        # ---- constants / inputs into SBUF ----
        msk_t = const.tile([P, 2 * P], F32)
        nc.gpsimd.dma_start(msk_t[:], msk_d[:])
        idn_t = const.tile([P, P], F32)
        nc.gpsimd.dma_start(idn_t[:], idn_d[:])
        km_t = const.tile([P, OB], F32)
        nc.gpsimd.dma_start(km_t[:], km_d.rearrange("(o p) -> p o", p=P))
        eps_t = const.tile([P, 1], F32)
        nc.vector.memset(eps_t[:], EPS)
        if has_b:
            bq_t = const.tile([P, KC], F32)
            nc.gpsimd.dma_start(bq_t[:], bq_d.rearrange("(c p) -> p c", p=P))
            bk_t = const.tile([P, KC], F32)
            nc.gpsimd.dma_start(bk_t[:], bk_d.rearrange("(c p) -> p c", p=P))
            bv_t = const.tile([1, HS], F32R)
            nc.gpsimd.dma_start(bv_t[:], bv_d[:])
            ones_f = const.tile([1, P], F32)
            nc.vector.memset(ones_f[:], 1.0)
            ones_t = const.tile([1, P], F32R)
            nc.vector.tensor_copy(ones_t[:], ones_f[:])

        hT_t = persist.tile([P, KC, SBAND], F32R)
        for c in range(KC):
            nc.gpsimd.dma_start(hT_t[:, c, :], hT_re[c])
        res_t = persist.tile([P, QB, HS], F32)
        nc.gpsimd.dma_start(res_t[:], res_d.rearrange("(q p) d -> p q d", p=P))

        qT_t = persist.tile([P, KC, SL], F32R)
        kT_t = persist.tile([P, KC, SBAND], F32R)
        v_t = persist.tile([P, OB, H, VST], F32R)
        out_t = persist.tile([P, QB, HS], F32)

        # ---- Q projection (dT-major): qT[m-chunk] = sum_c WqT[c,m].T @ hT[c, q-cols]
        wq_t = wpool.tile([P, KC, HS], F32R, tag="wfull")
        for c in range(KC):
            nc.gpsimd.dma_start(wq_t[:, c, :], wq_re[c])
        for m in range(KC):
            ps = pp_proj.tile([P, 512], F32, tag="proj")
            for c in range(KC):
                nc.tensor.matmul(ps[:], wq_t[:, c, m * P:(m + 1) * P],
                                 hT_t[:, c, P:P + SL],
                                 start=(c == 0), stop=(c == KC - 1))
            if has_b:
                nc.scalar.activation(qT_t[:, m, :], ps[:], AF.Identity,
                                     bias=bq_t[:, m:m + 1])
            else:
                nc.scalar.copy(qT_t[:, m, :], ps[:])

        # ---- K projection (dT-major over the full 768 band)
        wk_t = wpool.tile([P, KC, HS], F32R, tag="wfull")
        for c in range(KC):
            nc.gpsimd.dma_start(wk_t[:, c, :], wk_re[c])
        for m in range(KC):
            for n0, nn in ((0, 512), (512, 256)):
                ps = pp_proj.tile([P, 512], F32, tag="proj")
                for c in range(KC):
                    nc.tensor.matmul(ps[:, :nn], wk_t[:, c, m * P:(m + 1) * P],
                                     hT_t[:, c, n0:n0 + nn],
                                     start=(c == 0), stop=(c == KC - 1))
                if has_b:
                    nc.scalar.activation(kT_t[:, m, n0:n0 + nn], ps[:, :nn],
                                         AF.Identity, bias=bk_t[:, m:m + 1])
                else:
                    nc.scalar.copy(kT_t[:, m, n0:n0 + nn], ps[:, :nn])

        # ---- V projection (s-major, 8 heads per 512-wide psum)
        wv_t = wpool.tile([P, KC, HS], F32R, tag="wfull")
        for c in range(KC):
            nc.gpsimd.dma_start(wv_t[:, c, :], wv_re[c])
        onek_t = const.tile([P, OB * H * (VST - D)], F32)
        nc.vector.memset(onek_t[:], 1.0)
        nc.vector.tensor_copy(
            v_t[:, :, :, D:VST],
            onek_t[:].rearrange("p (o h t) -> p o h t", o=OB, h=H))
        for sb in range(OB):
            for n2 in range(2):
                ps = pp_proj.tile([P, 512], F32, tag="proj")
                for c in range(KC):
                    nc.tensor.matmul(ps[:], hT_t[:, c, sb * P:(sb + 1) * P],
                                     wv_t[:, c, n2 * 512:(n2 + 1) * 512],
                                     start=(c == 0),
                                     stop=(c == KC - 1) and not has_b)
                if has_b:
                    nc.tensor.matmul(ps[:], ones_t[:1, :],
                                     bv_t[:1, n2 * 512:(n2 + 1) * 512],
                                     start=False, stop=True)
                nc.scalar.copy(
                    v_t[:, sb, n2 * 8:(n2 + 1) * 8, 0:D],
                    ps[:].rearrange("p (h d) -> p h d", d=D))
        # fold key mask (edge padding + attention_mask factors) into v' rows,
        # including the denominator column
        for o in range(OB):
            nc.vector.tensor_scalar_mul(v_t[:, o, :, :], v_t[:, o, :, :],
                                        km_t[:, o:o + 1])

        # ---- attention per head ----
        for h in range(H):
            c_h = h // 2
            pb = (h % 2) * D
            ps_c = pp_ctx.tile([VST, 512], F32, tag="ctx")
            for o in range(OB):
                qb0 = max(0, o - 2)
                qb1 = min(QB - 1, o)
                ncols = (qb1 - qb0 + 1) * P
                ps_s = pp_sc.tile([P, 3 * P], F32, tag="sc")
                nc.tensor.matmul(ps_s[:, :ncols],
                                 kT_t[pb:pb + D, c_h, o * P:(o + 1) * P],
                                 qT_t[pb:pb + D, c_h, qb0 * P:qb0 * P + ncols],
                                 start=True, stop=True)
                pt = probs_p.tile([P, 3 * P], F32R, tag="probs")
                nc.scalar.activation(pt[:, :ncols], ps_s[:, :ncols], AF.Exp,
                                     scale=0.125)
                if o <= QB - 1:       # band mask M0 on the qb == o slice
                    pos = o - qb0
                    sl = pt[:, pos * P:(pos + 1) * P]
                    nc.vector.tensor_mul(sl, sl, msk_t[:, 0:P])
                if o >= 2:            # band mask M2 on the qb == o-2 slice
                    sl = pt[:, 0:P]
                    nc.vector.tensor_mul(sl, sl, msk_t[:, P:2 * P])
                nc.tensor.matmul(ps_c[:, qb0 * P:qb0 * P + ncols],
                                 v_t[:, o, h, :], pt[:, :ncols],
                                 start=(o == 0), stop=(o == OB - 1))
            cs = ctxsb_p.tile([VST, 512], F32, tag="ctxsb")
            nc.scalar.copy(cs[:], ps_c[:])
            for qb in range(QB):
                tp = pp_tr.tile([P, VST], F32, tag="tr")
                nc.tensor.transpose(tp[:], cs[:, qb * P:(qb + 1) * P],
                                    idn_t[:VST, :VST])
                rc = stats_p.tile([P, 1], F32, tag="rc")
                nc.vector.reciprocal(rc[:], tp[:, D:D + 1])
                # res += ctx * (1/denom)   (fused un-normalize + residual)
                nc.vector.scalar_tensor_tensor(
                    res_t[:, qb, h * D:(h + 1) * D], tp[:, 0:D], rc[:],
                    res_t[:, qb, h * D:(h + 1) * D],
                    op0=ALU.mult, op1=ALU.add)

        # ---- layernorm over each row of res ----
        for qb in range(QB):
            xq = res_t[:, qb, :]
            s1 = stats_p.tile([P, 1], F32, tag="st")
            nc.vector.tensor_reduce(s1[:], xq, axis=AX.X, op=ALU.add)
            s2a = stats_p.tile([P, 1], F32, tag="st")
            s2b = stats_p.tile([P, 1], F32, tag="st")
            sq0 = pp_proj.tile([P, 512], F32, tag="proj")
            nc.scalar.activation(sq0[:], res_t[:, qb, 0:512], AF.Square,
                                 accum_out=s2a[:])
            sq1 = pp_proj.tile([P, 512], F32, tag="proj")
            nc.scalar.activation(sq1[:], res_t[:, qb, 512:1024], AF.Square,
                                 accum_out=s2b[:])
            negmu = stats_p.tile([P, 1], F32, tag="st")
            nc.vector.tensor_scalar(negmu[:], s1[:], -1.0 / HS, None,
                                    op0=ALU.mult)
            e2 = stats_p.tile([P, 1], F32, tag="st")
            nc.vector.scalar_tensor_tensor(e2[:], s2a[:], 1.0, s2b[:],
                                           op0=ALU.mult, op1=ALU.add)
            var = stats_p.tile([P, 1], F32, tag="st")
            # var = (s2a+s2b)/HS - mu^2  ==  (e2*(1/HS)) - negmu*negmu
            mu2 = stats_p.tile([P, 1], F32, tag="st")
            nc.vector.tensor_mul(mu2[:], negmu[:], negmu[:])
            nc.vector.tensor_scalar(var[:], e2[:], 1.0 / HS, None,
                                    op0=ALU.mult)
            nc.vector.tensor_sub(var[:], var[:], mu2[:])
            std = stats_p.tile([P, 1], F32, tag="st")
            nc.scalar.activation(std[:], var[:], AF.Sqrt, bias=eps_t[:])
            rstd = stats_p.tile([P, 1], F32, tag="st")
            nc.vector.reciprocal(rstd[:], std[:])
            nmr = stats_p.tile([P, 1], F32, tag="st")
            nc.vector.tensor_mul(nmr[:], negmu[:], rstd[:])
            nc.scalar.activation(out_t[:, qb, :], xq, AF.Identity,
                                 bias=nmr[:], scale=rstd[:])
            if has_ln:
                gam_t = persist.tile([P, HS], F32, tag="gam")
                bet_t = persist.tile([P, HS], F32, tag="bet")
                if qb == 0:
                    nc.gpsimd.dma_start(gam_t[:], gam_d[:])
                    nc.gpsimd.dma_start(bet_t[:], bet_d[:])
                nc.vector.tensor_mul(out_t[:, qb, :], out_t[:, qb, :], gam_t[:])
                nc.vector.tensor_add(out_t[:, qb, :], out_t[:, qb, :], bet_t[:])

        nc.gpsimd.dma_start(out_d.rearrange("(q p) d -> p q d", p=P), out_t[:])

    nc.compile()
    return nc


_CACHE: dict = {}


def _get_nc(has_b: bool, has_ln: bool):
    key = (has_b, has_ln)
    if key not in _CACHE:
        _CACHE[key] = _build(*key)
    return _CACHE[key]


def _prep_inputs(hidden_states, attention_mask, Wq, bq, Wk, bk, Wv, bv,
                 ln_gamma, ln_beta):
    hs = np.asarray(hidden_states, dtype=np.float32)[0]      # [S, HS]
    am = np.asarray(attention_mask, dtype=np.float32)[0]     # [S]
    Wq = np.asarray(Wq, dtype=np.float32)
    Wk = np.asarray(Wk, dtype=np.float32)
    Wv = np.asarray(Wv, dtype=np.float32)
    bq = np.asarray(bq, dtype=np.float32)
    bk = np.asarray(bk, dtype=np.float32)
    bv = np.asarray(bv, dtype=np.float32)
    gam = np.asarray(ln_gamma, dtype=np.float32)
    bet = np.asarray(ln_beta, dtype=np.float32)

    has_b = bool(np.any(bq) or np.any(bk) or np.any(bv))
    has_ln = bool(np.any(gam != 1.0) or np.any(bet))

    hT = np.ascontiguousarray(hs.T)                          # [HS, S]
    wqT = np.ascontiguousarray(Wq.T)
    wkT = np.ascontiguousarray(Wk.T)
    wvT = np.ascontiguousarray(Wv.T)

    tri = np.tri(P, dtype=np.float32)
    masks = np.concatenate([tri, 1.0 - tri], axis=1)         # [P, 2P] = M0|M2
    iden = np.eye(P, dtype=np.float32)

    # key-mask factor per global key: exp(-10000 * (1 - mask_j)); exactly 1.0
    # for mask==1 and exactly 0.0 (fp32 underflow) for mask==0, matching the
    # reference's additive -10000 pre-softmax bias.
    kfac = np.exp(-10000.0 * (1.0 - am)).astype(np.float32)

    in_maps = []
    for c in range(N_CORES):
        lo = c * SL - P
        hi = c * SL + SL + P
        hT_band = np.zeros((HS, SBAND), dtype=np.float32)
        km = np.zeros((SBAND,), dtype=np.float32)
        s0, s1 = max(lo, 0), min(hi, S)
        hT_band[:, s0 - lo:s1 - lo] = hT[:, s0:s1]
        km[s0 - lo:s1 - lo] = kfac[s0:s1]
        m = {
            "hT": hT_band,
            "wqT": wqT, "wkT": wkT, "wvT": wvT,
            "res": hs[c * SL:(c + 1) * SL],
            "masks": masks, "iden": iden, "kmask": km,
        }
        if has_b:
            m["bq"] = bq
            m["bk"] = bk
            m["bv"] = bv.reshape(1, HS)
        if has_ln:
            m["gam"] = np.ascontiguousarray(np.broadcast_to(gam, (P, HS)))
            m["bet"] = np.ascontiguousarray(np.broadcast_to(bet, (P, HS)))
        in_maps.append(m)
    return in_maps, has_b, has_ln


def kernel(hidden_states, attention_mask, Wq, bq, Wk, bk, Wv, bv,
           ln_gamma, ln_beta):
    in_maps, has_b, has_ln = _prep_inputs(
        hidden_states, attention_mask, Wq, bq, Wk, bk, Wv, bv,
        ln_gamma, ln_beta)
    nc = _get_nc(has_b, has_ln)
    res = run_bass_kernel_spmd(nc, in_maps, list(range(N_CORES)))
    out = np.concatenate([res.results[c]["out"] for c in range(N_CORES)],
                         axis=0)
    return out.reshape(1, S, HS)


# revision 9
# speedup vs baseline: 6684.9649x; 6684.9649x over previous
"""Trainium2 Bass kernel for EnhancedAttentionV2:
sliding-window (256) attention + residual + layernorm, B=1, S=4096, HS=1024,
H=16 heads, D=64. Sequence-parallel across 8 NeuronCores: each core computes
512 query rows; K/V for its 768-key band (128-halo each side) are recomputed
locally from a zero-padded transposed slice of hidden_states, so no
collectives are needed.

Per-core device algorithm (all matmuls in float32r):
  qT[dout, s]  = WqT-chunks.T @ hT        (dT-major, 64-row head slices)
  kT[dout, j]  = WkT-chunks.T @ hT_band
  v'[j, h*66+.]= hT-chunks.T @ WvT        (s-major, per-head 66-wide slot:
                                           64 v cols | kmask col | pad)
  per (head, key-chunk o):  scoresT[j, q] = kT-slice.T @ qT-slice  (one matmul,
        q spans the <=3 query blocks whose window covers chunk o)
  probsT = exp(0.125 * scoresT)           (no max-subtraction; scores ~ N(0,1))
  band masks: tri-mask multiply on the 2 edge slices of each chunk
  ctxT'[66, q] += v'-slot.T @ probsT      (row 64 = softmax denominators via
                                           the kmask column riding in lhsT)
  transpose ctxT' per query block (PE transpose), then
  res += ctx * recip(denom)               (fused un-normalize + residual)
  layernorm rows of res -> out
"""

import os
from contextlib import ExitStack

import numpy as np

import concourse.bass as bass
import concourse.mybir as mybir
import concourse.tile as tile
from concourse import bacc

P = 128
S, HS, H, D = 4096, 1024, 16, 64
N_CORES = 8
SL = S // N_CORES           # 512 local query rows
QB = SL // P                # 4 query blocks
SBAND = SL + 2 * P          # 768 band keys
OB = SBAND // P             # 6 key chunks
KC = HS // P                # 8 contraction chunks
VST = 66                    # per-head slot width in v' (64 v | kmask | pad)
EPS = 1e-12
F32 = mybir.dt.float32
F32R = mybir.dt.float32r
AF = mybir.ActivationFunctionType
ALU = mybir.AluOpType
AX = mybir.AxisListType


def _build(has_b: bool, has_ln: bool):
    nc = bacc.Bacc(None, target_bir_lowering=False, debug=False,
                   num_devices=N_CORES)
    hT_d = nc.dram_tensor("hT", [HS, SBAND], F32R, kind="ExternalInput").ap()
    wq_d = nc.dram_tensor("wqT", [HS, HS], F32R, kind="ExternalInput").ap()
    wk_d = nc.dram_tensor("wkT", [HS, HS], F32R, kind="ExternalInput").ap()
    wv_d = nc.dram_tensor("wvT", [HS, HS], F32R, kind="ExternalInput").ap()
    res_d = nc.dram_tensor("res", [SL, HS], F32, kind="ExternalInput").ap()
    msk_d = nc.dram_tensor("masks", [P, 2 * P], F32, kind="ExternalInput").ap()
    idn_d = nc.dram_tensor("iden", [P, P], F32, kind="ExternalInput").ap()
    km_d = nc.dram_tensor("kmask", [SBAND], F32, kind="ExternalInput").ap()
    if has_b:
        bq_d = nc.dram_tensor("bq", [HS], F32, kind="ExternalInput").ap()
        bk_d = nc.dram_tensor("bk", [HS], F32, kind="ExternalInput").ap()
        bv_d = nc.dram_tensor("bv", [1, HS], F32R, kind="ExternalInput").ap()
    if has_ln:
        gam_d = nc.dram_tensor("gam", [P, HS], F32, kind="ExternalInput").ap()
        bet_d = nc.dram_tensor("bet", [P, HS], F32, kind="ExternalInput").ap()
    out_d = nc.dram_tensor("out", [SL, HS], F32, kind="ExternalOutput").ap()

    hT_re = hT_d.rearrange("(c p) j -> c p j", p=P)
    wq_re = wq_d.rearrange("(c p) d -> c p d", p=P)
    wk_re = wk_d.rearrange("(c p) d -> c p d", p=P)
    wv_re = wv_d.rearrange("(c p) d -> c p d", p=P)

    with tile.TileContext(nc) as tc, ExitStack() as ctx:
        const = ctx.enter_context(tc.tile_pool(name="const", bufs=1))
        wpool = ctx.enter_context(tc.tile_pool(name="wpool", bufs=2))
        persist = ctx.enter_context(tc.tile_pool(name="persist", bufs=1))
        probs_p = ctx.enter_context(tc.tile_pool(name="probs", bufs=8))
        ctxsb_p = ctx.enter_context(tc.tile_pool(name="ctxsb", bufs=2))
        stats_p = ctx.enter_context(tc.tile_pool(name="stats", bufs=48))
        pp_proj = ctx.enter_context(tc.tile_pool(name="ppproj", bufs=2, space="PSUM"))
        pp_sc = ctx.enter_context(tc.tile_pool(name="ppsc", bufs=2, space="PSUM"))
        pp_ctx = ctx.enter_context(tc.tile_pool(name="ppctx", bufs=2, space="PSUM"))
        pp_tr = ctx.enter_context(tc.tile_pool(name="pptr", bufs=2, space="PSUM"))

        # ---- constants / inputs into SBUF ----
        msk_t = const.tile([P, 2 * P], F32)
        nc.gpsimd.dma_start(msk_t[:], msk_d[:])
        idn_t = const.tile([P, P], F32)
        nc.gpsimd.dma_start(idn_t[:], idn_d[:])
        km_t = const.tile([P, OB], F32)
        nc.gpsimd.dma_start(km_t[:], km_d.rearrange("(o p) -> p o", p=P))
        eps_t = const.tile([P, 1], F32)
        nc.vector.memset(eps_t[:], EPS)
        if has_b:
            bq_t = const.tile([P, KC], F32)
            nc.gpsimd.dma_start(bq_t[:], bq_d.rearrange("(c p) -> p c", p=P))
            bk_t = const.tile([P, KC], F32)
            nc.gpsimd.dma_start(bk_t[:], bk_d.rearrange("(c p) -> p c", p=P))
            bv_t = const.tile([1, HS], F32R)
            nc.gpsimd.dma_start(bv_t[:], bv_d[:])
            ones_f = const.tile([1, P], F32)
            nc.vector.memset(ones_f[:], 1.0)
            ones_t = const.tile([1, P], F32R)
            nc.vector.tensor_copy(ones_t[:], ones_f[:])

        hT_t = persist.tile([P, KC, SBAND], F32R)
        for c in range(KC):
            nc.gpsimd.dma_start(hT_t[:, c, :], hT_re[c])
        res_t = persist.tile([P, QB, HS], F32)
        nc.gpsimd.dma_start(res_t[:], res_d.rearrange("(q p) d -> p q d", p=P))

        qT_t = persist.tile([P, KC, SL], F32R)
        kT_t = persist.tile([P, KC, SBAND], F32R)
        v_t = persist.tile([P, OB, H, VST], F32R)
        out_t = persist.tile([P, QB, HS], F32)

        # ---- Q projection (dT-major): qT[m-chunk] = sum_c WqT[c,m].T @ hT[c, q-cols]
        wq_t = wpool.tile([P, KC, HS], F32R, tag="wfull")
        for c in range(KC):
            nc.gpsimd.dma_start(wq_t[:, c, :], wq_re[c])
        for m in range(KC):
            ps = pp_proj.tile([P, 512], F32, tag="proj")
            for c in range(KC):
                nc.tensor.matmul(ps[:], wq_t[:, c, m * P:(m + 1) * P],
                                 hT_t[:, c, P:P + SL],
                                 start=(c == 0), stop=(c == KC - 1))
            if has_b:
                nc.scalar.activation(qT_t[:, m, :], ps[:], AF.Identity,
                                     bias=bq_t[:, m:m + 1])
            else:
                nc.scalar.copy(qT_t[:, m, :], ps[:])

        # ---- K projection (dT-major over the full 768 band)
        wk_t = wpool.tile([P, KC, HS], F32R, tag="wfull")
        for c in range(KC):
            nc.gpsimd.dma_start(wk_t[:, c, :], wk_re[c])
        for m in range(KC):
            for n0, nn in ((0, 512), (512, 256)):
                ps = pp_proj.tile([P, 512], F32, tag="proj")
                for c in range(KC):
                    nc.tensor.matmul(ps[:, :nn], wk_t[:, c, m * P:(m + 1) * P],
                                     hT_t[:, c, n0:n0 + nn],
                                     start=(c == 0), stop=(c == KC - 1))
                if has_b:
                    nc.scalar.activation(kT_t[:, m, n0:n0 + nn], ps[:, :nn],
                                         AF.Identity, bias=bk_t[:, m:m + 1])
                else:
                    nc.scalar.copy(kT_t[:, m, n0:n0 + nn], ps[:, :nn])

        # ---- V projection (s-major, 8 heads per 512-wide psum)
        wv_t = wpool.tile([P, KC, HS], F32R, tag="wfull")
        for c in range(KC):
            nc.gpsimd.dma_start(wv_t[:, c, :], wv_re[c])
        onek_t = const.tile([P, OB * H * (VST - D)], F32)
        nc.vector.memset(onek_t[:], 1.0)
        nc.vector.tensor_copy(
            v_t[:, :, :, D:VST],
            onek_t[:].rearrange("p (o h t) -> p o h t", o=OB, h=H))
        for sb in range(OB):
            for n2 in range(2):
                ps = pp_proj.tile([P, 512], F32, tag="proj")
                for c in range(KC):
                    nc.tensor.matmul(ps[:], hT_t[:, c, sb * P:(sb + 1) * P],
                                     wv_t[:, c, n2 * 512:(n2 + 1) * 512],
                                     start=(c == 0),
                                     stop=(c == KC - 1) and not has_b)
                if has_b:
                    nc.tensor.matmul(ps[:], ones_t[:1, :],
                                     bv_t[:1, n2 * 512:(n2 + 1) * 512],
                                     start=False, stop=True)
                nc.scalar.copy(
                    v_t[:, sb, n2 * 8:(n2 + 1) * 8, 0:D],
                    ps[:].rearrange("p (h d) -> p h d", d=D))
        # fold key mask (edge padding + attention_mask factors) into v' rows,
        # including the denominator column
        for o in range(OB):
            nc.vector.tensor_scalar_mul(v_t[:, o, :, :], v_t[:, o, :, :],
                                        km_t[:, o:o + 1])

        # ---- attention per head ----
        for h in range(H):
            c_h = h // 2
            pb = (h % 2) * D
            ps_c = pp_ctx.tile([VST, 512], F32, tag="ctx")
            for o in range(OB):
                qb0 = max(0, o - 2)
                qb1 = min(QB - 1, o)
                ncols = (qb1 - qb0 + 1) * P
                ps_s = pp_sc.tile([P, 3 * P], F32, tag="sc")
                nc.tensor.matmul(ps_s[:, :ncols],
                                 kT_t[pb:pb + D, c_h, o * P:(o + 1) * P],
                                 qT_t[pb:pb + D, c_h, qb0 * P:qb0 * P + ncols],
                                 start=True, stop=True)
                pt = probs_p.tile([P, 3 * P], F32R, tag="probs")
                nc.scalar.activation(pt[:, :ncols], ps_s[:, :ncols], AF.Exp,
                                     scale=0.125)
                if o <= QB - 1:       # band mask M0 on the qb == o slice
                    pos = o - qb0
                    sl = pt[:, pos * P:(pos + 1) * P]
                    nc.vector.tensor_mul(sl, sl, msk_t[:, 0:P])
                if o >= 2:            # band mask M2 on the qb == o-2 slice
                    sl = pt[:, 0:P]
                    nc.vector.tensor_mul(sl, sl, msk_t[:, P:2 * P])
                nc.tensor.matmul(ps_c[:, qb0 * P:qb0 * P + ncols],
                                 v_t[:, o, h, :], pt[:, :ncols],
                                 start=(o == 0), stop=(o == OB - 1))
            cs = ctxsb_p.tile([VST, 512], F32, tag="ctxsb")
            nc.scalar.copy(cs[:], ps_c[:])
            for qb in range(QB):
                tp = pp_tr.tile([P, VST], F32, tag="tr")
                nc.tensor.transpose(tp[:], cs[:, qb * P:(qb + 1) * P],
                                    idn_t[:VST, :VST])
                rc = stats_p.tile([P, 1], F32, tag="rc")
                nc.vector.reciprocal(rc[:], tp[:, D:D + 1])
                # res += ctx * (1/denom)   (fused un-normalize + residual)
                nc.vector.scalar_tensor_tensor(
                    res_t[:, qb, h * D:(h + 1) * D], tp[:, 0:D], rc[:],
                    res_t[:, qb, h * D:(h + 1) * D],
                    op0=ALU.mult, op1=ALU.add)

        # ---- layernorm over each row of res ----
        for qb in range(QB):
            xq = res_t[:, qb, :]
            s1 = stats_p.tile([P, 1], F32, tag="st")
            nc.vector.tensor_reduce(s1[:], xq, axis=AX.X, op=ALU.add)
            s2a = stats_p.tile([P, 1], F32, tag="st")
            s2b = stats_p.tile([P, 1], F32, tag="st")
            sq0 = pp_proj.tile([P, 512], F32, tag="proj")
            nc.scalar.activation(sq0[:], res_t[:, qb, 0:512], AF.Square,
                                 accum_out=s2a[:])
            sq1 = pp_proj.tile([P, 512], F32, tag="proj")
            nc.scalar.activation(sq1[:], res_t[:, qb, 512:1024], AF.Square,
                                 accum_out=s2b[:])
            negmu = stats_p.tile([P, 1], F32, tag="st")
            nc.vector.tensor_scalar(negmu[:], s1[:], -1.0 / HS, None,
                                    op0=ALU.mult)
            e2 = stats_p.tile([P, 1], F32, tag="st")
            nc.vector.scalar_tensor_tensor(e2[:], s2a[:], 1.0, s2b[:],
                                           op0=ALU.mult, op1=ALU.add)
            var = stats_p.tile([P, 1], F32, tag="st")
            # var = (s2a+s2b)/HS - mu^2  ==  (e2*(1/HS)) - negmu*negmu
            mu2 = stats_p.tile([P, 1], F32, tag="st")
            nc.vector.tensor_mul(mu2[:], negmu[:], negmu[:])
            nc.vector.tensor_scalar(var[:], e2[:], 1.0 / HS, None,
                                    op0=ALU.mult)
            nc.vector.tensor_sub(var[:], var[:], mu2[:])
            std = stats_p.tile([P, 1], F32, tag="st")
            nc.scalar.activation(std[:], var[:], AF.Sqrt, bias=eps_t[:])
            rstd = stats_p.tile([P, 1], F32, tag="st")
            nc.vector.reciprocal(rstd[:], std[:])
            nmr = stats_p.tile([P, 1], F32, tag="st")
            nc.vector.tensor_mul(nmr[:], negmu[:], rstd[:])
            nc.scalar.activation(out_t[:, qb, :], xq, AF.Identity,
                                 bias=nmr[:], scale=rstd[:])
            if has_ln:
                gam_t = persist.tile([P, HS], F32, tag="gam")
                bet_t = persist.tile([P, HS], F32, tag="bet")
                if qb == 0:
                    nc.gpsimd.dma_start(gam_t[:], gam_d[:])
                    nc.gpsimd.dma_start(bet_t[:], bet_d[:])
                nc.vector.tensor_mul(out_t[:, qb, :], out_t[:, qb, :], gam_t[:])
                nc.vector.tensor_add(out_t[:, qb, :], out_t[:, qb, :], bet_t[:])

        nc.gpsimd.dma_start(out_d.rearrange("(q p) d -> p q d", p=P), out_t[:])

    nc.compile()
    return nc


class _Runner:
    """Reusable jitted SPMD executor for a compiled Bass program.

    Mirrors concourse.bass2jax.run_bass_via_pjrt's multi-core path, but keeps
    the jitted function and device-resident inputs so repeat executions skip
    lowering/compile and host->device staging.
    """

    def __init__(self, nc):
        import jax
        from jax.experimental.shard_map import shard_map
        from jax.sharding import Mesh, NamedSharding, PartitionSpec
        from concourse import bass2jax

        bass2jax.install_neuronx_cc_hook()
        self.nc = nc
        in_names: list[str] = []
        out_names: list[str] = []
        out_avals = []
        zero_outs: list[np.ndarray] = []
        partition_name = (nc.partition_id_tensor.name
                          if nc.partition_id_tensor else None)
        for alloc in nc.m.functions[0].allocations:
            if not isinstance(alloc, mybir.MemoryLocationSet):
                continue
            name = alloc.memorylocations[0].name
            if alloc.kind == "ExternalInput":
                if name != partition_name:
                    in_names.append(name)
            elif alloc.kind == "ExternalOutput":
                shape = tuple(alloc.tensor_shape)
                dtype = mybir.dt.np(alloc.dtype)
                out_names.append(name)
                out_avals.append(jax.core.ShapedArray(shape, dtype))
                zero_outs.append(np.zeros(shape, dtype))
        self.n_params = len(in_names)
        self.in_names = list(in_names)
        self.out_names = out_names
        self.out_avals = out_avals
        self.zero_outs = zero_outs
        all_in_names = in_names + out_names
        if partition_name is not None:
            all_in_names.append(partition_name)

        def _body(*args):
            operands = list(args)
            if partition_name is not None:
                operands.append(bass2jax.partition_id_tensor())
            outs = bass2jax._bass_exec_p.bind(
                *operands,
                out_avals=tuple(out_avals),
                in_names=tuple(all_in_names),
                out_names=tuple(out_names),
                lowering_input_output_aliases=(),
                sim_require_finite=True,
                sim_require_nnan=True,
                nc=nc,
            )
            return tuple(outs)

        devices = jax.devices()[:N_CORES]
        self.mesh = Mesh(np.asarray(devices), ("core",))
        self.sharding = NamedSharding(self.mesh, PartitionSpec("core"))
        n_all = self.n_params + len(out_names)
        self.fn = jax.jit(
            shard_map(_body, mesh=self.mesh,
                      in_specs=(PartitionSpec("core"),) * n_all,
                      out_specs=(PartitionSpec("core"),) * len(out_names),
                      check_rep=False),
            keep_unused=True,
        )

    def stage(self, in_maps):
        import jax
        args = []
        for i, name in enumerate(self.in_names):
            concat = np.concatenate(
                [np.asarray(m[name]) for m in in_maps], axis=0)
            args.append(jax.device_put(concat, self.sharding))
        for z in self.zero_outs:
            zz = np.zeros((N_CORES * z.shape[0], *z.shape[1:]), z.dtype)
            args.append(jax.device_put(zz, self.sharding))
        return args

    def run(self, staged):
        out_arrs = self.fn(*staged)
        return [a.block_until_ready() for a in out_arrs]

    def results(self, out_arrs):
        res = []
        for c in range(N_CORES):
            res.append({
                name: np.asarray(out_arrs[i]).reshape(
                    N_CORES, *self.out_avals[i].shape)[c]
                for i, name in enumerate(self.out_names)
            })
        return res


_CACHE: dict = {}


def _get_runner(has_b: bool, has_ln: bool) -> _Runner:
    key = (has_b, has_ln)
    if key not in _CACHE:
        _CACHE[key] = _Runner(_build(*key))
    return _CACHE[key]


def _prep_inputs(hidden_states, attention_mask, Wq, bq, Wk, bk, Wv, bv,
                 ln_gamma, ln_beta):
    hs = np.asarray(hidden_states, dtype=np.float32)[0]      # [S, HS]
    am = np.asarray(attention_mask, dtype=np.float32)[0]     # [S]
    Wq = np.asarray(Wq, dtype=np.float32)
    Wk = np.asarray(Wk, dtype=np.float32)
    Wv = np.asarray(Wv, dtype=np.float32)
    bq = np.asarray(bq, dtype=np.float32)
    bk = np.asarray(bk, dtype=np.float32)
    bv = np.asarray(bv, dtype=np.float32)
    gam = np.asarray(ln_gamma, dtype=np.float32)
    bet = np.asarray(ln_beta, dtype=np.float32)

    has_b = bool(np.any(bq) or np.any(bk) or np.any(bv))
    has_ln = bool(np.any(gam != 1.0) or np.any(bet))

    hT = np.ascontiguousarray(hs.T)                          # [HS, S]
    wqT = np.ascontiguousarray(Wq.T)
    wkT = np.ascontiguousarray(Wk.T)
    wvT = np.ascontiguousarray(Wv.T)

    tri = np.tri(P, dtype=np.float32)
    masks = np.concatenate([tri, 1.0 - tri], axis=1)         # [P, 2P] = M0|M2
    iden = np.eye(P, dtype=np.float32)

    # key-mask factor per global key: exp(-10000 * (1 - mask_j)); exactly 1.0
    # for mask==1 and exactly 0.0 (fp32 underflow) for mask==0, matching the
    # reference's additive -10000 pre-softmax bias.
    kfac = np.exp(-10000.0 * (1.0 - am)).astype(np.float32)

    in_maps = []
    for c in range(N_CORES):
        lo = c * SL - P
        hi = c * SL + SL + P
        hT_band = np.zeros((HS, SBAND), dtype=np.float32)
        km = np.zeros((SBAND,), dtype=np.float32)
        s0, s1 = max(lo, 0), min(hi, S)
        hT_band[:, s0 - lo:s1 - lo] = hT[:, s0:s1]
        km[s0 - lo:s1 - lo] = kfac[s0:s1]
        m = {
            "hT": hT_band,
            "wqT": wqT, "wkT": wkT, "wvT": wvT,
            "res": hs[c * SL:(c + 1) * SL],
            "masks": masks, "iden": iden, "kmask": km,
        }
        if has_b:
            m["bq"] = bq
            m["bk"] = bk
            m["bv"] = bv.reshape(1, HS)
        if has_ln:
            m["gam"] = np.ascontiguousarray(np.broadcast_to(gam, (P, HS)))
            m["bet"] = np.ascontiguousarray(np.broadcast_to(bet, (P, HS)))
        in_maps.append(m)
    return in_maps, has_b, has_ln


def kernel(hidden_states, attention_mask, Wq, bq, Wk, bk, Wv, bv,
           ln_gamma, ln_beta):
    in_maps, has_b, has_ln = _prep_inputs(
        hidden_states, attention_mask, Wq, bq, Wk, bk, Wv, bv,
        ln_gamma, ln_beta)
    runner = _get_runner(has_b, has_ln)
    staged = runner.stage(in_maps)
    res = runner.results(runner.run(staged))
    out = np.concatenate([res[c]["out"] for c in range(N_CORES)], axis=0)
    return out.reshape(1, S, HS)
